# revision 39
# baseline (speedup 1.0000x reference)
"""Bass/Tile TRN2 kernel for nn_AttentionLayer (additive attention).

reference:
    q_proj = query @ W_q + b_q                  # [B, A]
    k_proj = keys @ W_k + b_k                   # [B, S, A]
    qk     = tanh(q_proj[:, None, :] + k_proj)  # [B, S, A]
    scores = qk @ w_e                           # [B, S]
    scores = where(mask == 0, -1e9, scores)
    attn   = softmax(scores, axis=1)
    out    = einsum("bs,bse->be", attn, values) # [B, E]

Sharding: pure data parallel over batch across 8 cores (512 batches/core).

Per-core layout: slabs of 32 batches (6400 rows of (b, s)). Rows are
partition-major: partition p holds rows [p*50, (p+1)*50) of the slab, so
batch = p // 4 is constant per partition and each slab DMA is fully
contiguous per partition line.

Pipeline per slab:
  keys/values DMA (fp32->bf16 cast in SWDGE) -> PE transpose of keys tiles
  -> k_proj matmul (W_k stationary) + q-term matmul (one-hot replication)
  -> tanh on ACT (per-partition bias = b_q + b_k, PSUM->SBUF fused)
  -> scores matmul (qk stationary, [w_e;0|0;w_e] pair trick) -> [row, j] PSUM
  -> mask add, exp on ACT with accum_out (softmax denominator for free)
  -> attn_sel = onehot * attn (broadcast) -> context matmul (attn_sel
     stationary, values streaming) -> normalize via ACT per-partition scale.
"""

from contextlib import ExitStack

import numpy as np
import ml_dtypes

import concourse.bass as bass
import concourse.bacc as bacc
import concourse.tile as tile
import concourse.mybir as mybir
from concourse import bass_utils

BF16 = mybir.dt.bfloat16
F32 = mybir.dt.float32
I32 = mybir.dt.int32

EMBED = 128
ATT = 64
SEQ = 200
SLAB_B = 32          # batches per slab
J = 50               # free columns per partition per slab (SLAB_B*SEQ/128)
NPAIR = J // 2       # 25 j-pairs per slab
HALVES = [(0, 13), (13, 12)]  # (pair offset, npairs) per half-slab

NEG_BIG = -1.0e9


def _np_consts(W_q, b_q, W_k, b_k, w_e):
    bf = ml_dtypes.bfloat16
    ident = np.eye(128, dtype=bf)
    oh4 = np.zeros((SLAB_B, 128), dtype=bf)          # [b, p] = (p//4 == b)
    for p in range(128):
        oh4[p // 4, p] = 1.0
    oh4T_bf = np.ascontiguousarray(oh4.T)            # [128, 32]
    oh4T_f32 = oh4T_bf.astype(np.float32)
    W_k = np.asarray(W_k, np.float32)
    wk_hi = W_k.astype(bf)
    wk_lo = (W_k - wk_hi.astype(np.float32)).astype(bf)
    w_e = np.asarray(w_e, np.float32)
    we_hi = w_e.astype(bf)
    we_lo = (w_e - we_hi.astype(np.float32)).astype(bf)
    # scores matmul rhs: cols = [hi_even, hi_odd, lo_even, lo_odd]
    we4 = np.zeros((128, 4), dtype=bf)
    we4[:ATT, 0] = we_hi
    we4[ATT:, 1] = we_hi
    we4[:ATT, 2] = we_lo
    we4[ATT:, 3] = we_lo
    biasqk = np.tile((b_q + b_k).astype(np.float32), 2).reshape(128, 1)
    return {
        "wk_c": np.ascontiguousarray(wk_hi),               # [128, 64]
        "wklo_c": np.ascontiguousarray(wk_lo),             # [128, 64]
        "wq_c": np.ascontiguousarray(W_q.astype(np.float32)),  # [128, 64] f32
        "ident_c": ident,                                   # [128, 128]
        "identf_c": np.eye(128, dtype=np.float32),          # [128, 128] f32
        "oh4_c": oh4,                                       # [32, 128]
        "oh4t_c": oh4T_bf,                                  # [128, 32]
        "oh4tf_c": oh4T_f32,                                # [128, 32] f32
        "we4_c": we4,                                       # [128, 4]
        "biasqk_c": biasqk,                                 # [128, 1] f32
    }


def build_program(b_core, num_devices, stage=99, repeat=1):
    """Build the Bass program for one core's shard of b_core batches.

    stage < 99 builds a truncated pipeline (debug bisection):
      0 input DMAs only, 1 +setup, 2 +transpose, 3 +kproj/tanh, 4 +scores,
      5 +exp/den/attn_sel, 99 full.
    repeat > 1 re-runs the whole slab pipeline (timing differencing).
    """
    assert b_core % SLAB_B == 0
    n_slabs = b_core // SLAB_B

    nc = bacc.Bacc(
        "TRN2",
        target_bir_lowering=False,
        debug=False,
        enable_asserts=True,
        num_devices=num_devices,
    )

    query_d = nc.dram_tensor("query", [b_core, EMBED], F32, kind="ExternalInput").ap()
    keys_d = nc.dram_tensor("keys", [b_core, SEQ, EMBED], F32, kind="ExternalInput").ap()
    values_d = nc.dram_tensor("values", [b_core, SEQ, EMBED], F32, kind="ExternalInput").ap()
    mask_d = nc.dram_tensor("mask", [b_core, SEQ], I32, kind="ExternalInput").ap()

    wk_d = nc.dram_tensor("wk_c", [EMBED, ATT], BF16, kind="ExternalInput").ap()
    wklo_d = nc.dram_tensor("wklo_c", [EMBED, ATT], BF16, kind="ExternalInput").ap()
    wq_d = nc.dram_tensor("wq_c", [EMBED, ATT], F32, kind="ExternalInput").ap()
    ident_d = nc.dram_tensor("ident_c", [128, 128], BF16, kind="ExternalInput").ap()
    identf_d = nc.dram_tensor("identf_c", [128, 128], F32, kind="ExternalInput").ap()
    oh4_d = nc.dram_tensor("oh4_c", [SLAB_B, 128], BF16, kind="ExternalInput").ap()
    oh4t_d = nc.dram_tensor("oh4t_c", [128, SLAB_B], BF16, kind="ExternalInput").ap()
    oh4tf_d = nc.dram_tensor("oh4tf_c", [128, SLAB_B], F32, kind="ExternalInput").ap()
    we4_d = nc.dram_tensor("we4_c", [128, 4], BF16, kind="ExternalInput").ap()
    biasqk_d = nc.dram_tensor("biasqk_c", [128, 1], F32, kind="ExternalInput").ap()

    out_d = nc.dram_tensor("out", [b_core, EMBED], F32, kind="ExternalOutput").ap()

    keys_flat = keys_d.rearrange("b s e -> (b s) e")
    values_flat = values_d.rearrange("b s e -> (b s) e")
    mask_flat = mask_d.rearrange("b s -> (b s)")

    Tanh = mybir.ActivationFunctionType.Tanh
    Exp = mybir.ActivationFunctionType.Exp
    Copy = mybir.ActivationFunctionType.Copy

    with tile.TileContext(nc) as tc, ExitStack() as ctx:
        singles = ctx.enter_context(tc.tile_pool(name="singles", bufs=1))
        inpool = ctx.enter_context(tc.tile_pool(name="inpool", bufs=2))
        ktpool = ctx.enter_context(tc.tile_pool(name="ktpool", bufs=2))
        qkpool = ctx.enter_context(tc.tile_pool(name="qkpool", bufs=2))
        smalls = ctx.enter_context(tc.tile_pool(name="smalls", bufs=2))
        outpool = ctx.enter_context(tc.tile_pool(name="outpool", bufs=2))
        # PSUM pools
        ktps_pool = ctx.enter_context(tc.tile_pool(name="ktps", bufs=2, space="PSUM"))
        qkps_pool = ctx.enter_context(tc.tile_pool(name="qkps", bufs=1, space="PSUM"))
        scps_pool = ctx.enter_context(tc.tile_pool(name="scps", bufs=1, space="PSUM"))
        ctxps_pool = ctx.enter_context(tc.tile_pool(name="ctxps", bufs=1, space="PSUM"))

        # ---- constants to SBUF ----
        wk_sb = singles.tile([EMBED, ATT], BF16, tag="wk")
        nc.sync.dma_start(out=wk_sb, in_=wk_d)
        wklo_sb = singles.tile([EMBED, ATT], BF16, tag="wklo")
        nc.sync.dma_start(out=wklo_sb, in_=wklo_d)
        wq_sb = singles.tile([EMBED, ATT], F32, tag="wq")
        nc.sync.dma_start(out=wq_sb, in_=wq_d)
        ident_sb = singles.tile([128, 128], BF16, tag="ident")
        nc.sync.dma_start(out=ident_sb, in_=ident_d)
        identf_sb = singles.tile([128, 128], F32, tag="identf")
        nc.sync.dma_start(out=identf_sb, in_=identf_d)
        oh4_sb = singles.tile([SLAB_B, 128], BF16, tag="oh4")
        nc.sync.dma_start(out=oh4_sb, in_=oh4_d)
        oh4t_sb = singles.tile([128, SLAB_B], BF16, tag="oh4t")
        nc.sync.dma_start(out=oh4t_sb, in_=oh4t_d)
        oh4tf_sb = singles.tile([128, SLAB_B], F32, tag="oh4tf")
        nc.sync.dma_start(out=oh4tf_sb, in_=oh4tf_d)
        we4_sb = singles.tile([128, 4], BF16, tag="we4")
        nc.sync.dma_start(out=we4_sb, in_=we4_d)
        biasqk_sb = singles.tile([128, 1], F32, tag="biasqk")
        nc.sync.dma_start(out=biasqk_sb, in_=biasqk_d)

        def standin_out(i, src_ap):
            """Debug stages: park some intermediate in the output tensor."""
            w = min(src_ap.shape[-1], EMBED)
            t = outpool.tile([SLAB_B, EMBED], F32, tag="ctx_sb")
            nc.vector.memset(t, 0.0)
            nc.scalar.copy(out=t[: src_ap.shape[0], :w], in_=src_ap[:, :w])
            nc.sync.dma_start(out=out_d[i * SLAB_B : (i + 1) * SLAB_B, :], in_=t)

        # ---- setup: queryT (bf16, [E, b_core]) and per-slab q_proj ----
        # q_all[b_local, i, a] = (query[32i + b_local] @ W_q)  (bias folded into tanh)
        n_qtiles = (b_core + 127) // 128
        queryT_sb = singles.tile([128, n_qtiles * 128], F32, tag="queryT")
        q_all_sb = None
        if stage >= 1:
            for r in range(n_qtiles):
                rows = min(128, b_core - r * 128)
                qa = smalls.tile([128, 128], F32, tag="qload")
                nc.sync.dma_start(out=qa[:rows, :], in_=query_d[r * 128 : r * 128 + rows, :])
                qt_ps = ktps_pool.tile([128, 4, 128], F32, tag="ktps")
                # fp32 transpose as a regular matmul (walrus rejects fp32
                # transpose-mode): out = qa.T @ I
                nc.tensor.matmul(
                    qt_ps[:, 0, :rows],
                    qa[:rows, :],
                    identf_sb[:rows, :rows],
                    start=True,
                    stop=True,
                )
                nc.vector.tensor_copy(
                    out=queryT_sb[:, r * 128 : r * 128 + rows], in_=qt_ps[:, 0, :rows]
                )
            q_all_sb = singles.tile([SLAB_B, n_slabs, ATT], BF16, tag="q_all")
            for i in range(n_slabs):
                qs_ps = scps_pool.tile([128, 64], F32, tag="scps")
                nc.tensor.matmul(
                    qs_ps[:SLAB_B, :ATT],
                    queryT_sb[:, i * SLAB_B : (i + 1) * SLAB_B],
                    wq_sb,
                    start=True,
                    stop=True,
                )
                nc.scalar.copy(out=q_all_sb[:, i, :], in_=qs_ps[:SLAB_B, :ATT])

        # ---- main loop over slabs ----
        for i in [i for _ in range(repeat) for i in range(n_slabs)]:
            r0 = i * SLAB_B * SEQ  # first flat row of slab

            keys_s = inpool.tile([128, J, EMBED], BF16, tag="keys_s")
            keys_slab = keys_flat[r0 : r0 + SLAB_B * SEQ, :].rearrange(
                "(p j) e -> p j e", p=128
            )
            # split so the first transposes can start at half the DMA
            half_j = J // 2
            nc.gpsimd.dma_start(
                out=keys_s[:, :half_j, :], in_=keys_slab[:, :half_j, :]
            )
            nc.gpsimd.dma_start(
                out=keys_s[:, half_j:, :], in_=keys_slab[:, half_j:, :]
            )
            vals_s = inpool.tile([128, J, EMBED], BF16, tag="vals_s")
            nc.gpsimd.dma_start(
                out=vals_s,
                in_=values_flat[r0 : r0 + SLAB_B * SEQ, :].rearrange(
                    "(p j) e -> p j e", p=128
                ),
            )
            mask_s = smalls.tile([128, J], I32, tag="mask_s")
            nc.sync.dma_start(
                out=mask_s,
                in_=mask_flat[r0 : r0 + SLAB_B * SEQ].rearrange("(p j) -> p j", p=128),
            )

            if stage < 2:
                standin_out(i, vals_s[0:SLAB_B, 0, :])
                continue

            # phase T: transpose keys -> keysT[e, j, p]
            keysT_sb = ktpool.tile([128, J, 128], BF16, tag="keysT")
            for g0 in range(0, J, 8):
                gn = min(8, J - g0)
                kt_ps = ktps_pool.tile([128, 8, 128], BF16, tag="ktps")
                for jj in range(gn):
                    # one bank per group: only the first transpose zeroes it
                    nc.tensor.matmul(
                        kt_ps[:, jj, :],
                        keys_s[:, g0 + jj, :],
                        ident_sb,
                        is_transpose=True,
                        start=(jj == 0),
                        stop=(jj == gn - 1),
                        skip_group_check=True,
                    )
                nc.vector.tensor_copy(
                    out=keysT_sb[:, g0 : g0 + gn, :], in_=kt_ps[:, :gn, :]
                )

            if stage < 3:
                standin_out(i, keysT_sb[0:SLAB_B, 0, :])
                continue

            # 4 cols per pair (hi/lo x even/odd) + a denominator col
            scores_ps = scps_pool.tile([128, 104], F32, tag="scps")

            for h0, hn in HALVES:
                # phase K: k_proj + q-inject, paired (j even -> partitions 0:64,
                # j odd -> 64:128)
                # PSUM start=True zeroes a whole 2KB bank (4 t-blocks): only
                # the first matmul touching each bank may set start=True.
                # Merge 4 t-blocks (one bank) per matmul: N=512, rhs strides
                # over the even (or odd) j slices of keysT.
                qk_ps = qkps_pool.tile([128, 13, 128], F32, tag="qkps")
                for wmat, first in ((wk_sb, True), (wklo_sb, False)):
                    for base, par in ((0, 0), (ATT, 1)):
                        for t0 in range(0, hn, 4):
                            tn = min(4, hn - t0)
                            rhs = keysT_sb.rearrange("p (t q) c -> p t q c", q=2)[
                                :, h0 + t0 : h0 + t0 + tn, par, :
                            ]
                            nc.tensor.matmul(
                                qk_ps[base : base + ATT, t0 : t0 + tn, :],
                                wmat,
                                rhs,
                                start=first,
                                stop=False,
                                skip_group_check=True,
                            )
                # q-inject: rhs broadcasts oh4 over the t-blocks of a bank
                for base in (0, ATT):
                    for t0 in range(0, hn, 4):
                        tn = min(4, hn - t0)
                        rhs = oh4_sb.unsqueeze(1).broadcast_to([SLAB_B, tn, 128])
                        nc.tensor.matmul(
                            qk_ps[base : base + ATT, t0 : t0 + tn, :],
                            q_all_sb[:, i, :],
                            rhs,
                            start=False,
                            stop=True,
                            skip_group_check=True,
                        )

                # tanh (bias = b_q + b_k), PSUM -> SBUF, f32 -> bf16
                qk_sb = qkpool.tile([128, 13, 128], BF16, tag="qk_sb")
                nc.scalar.activation(
                    out=qk_sb[:, :hn, :].rearrange("p t c -> p (t c)"),
                    in_=qk_ps[:, :hn, :].rearrange("p t c -> p (t c)"),
                    func=Tanh,
                    bias=biasqk_sb,
                    scale=1.0,
                )

                if stage < 4:
                    continue
                # phase S: scores for both parities of each pair, hi and lo
                # w_e halves, in one N=4 matmul. One shared bank: only the
                # very first write zeroes it.
                for t in range(hn):
                    tg = h0 + t
                    nc.tensor.matmul(
                        scores_ps[:, 4 * tg : 4 * tg + 4],
                        qk_sb[:, t, :],
                        we4_sb,
                        start=(tg == 0),
                        stop=(tg == NPAIR - 1),
                        skip_group_check=True,
                    )

            if stage < 4:
                standin_out(i, qk_sb[0:SLAB_B, 0, :])
                continue
            if stage < 5:
                standin_out(i, scores_ps[0:SLAB_B, 0:J])
                continue

            # phase M: mask, exp (+ denominator via accum_out)
            maskf = smalls.tile([128, J], F32, tag="maskf")
            nc.vector.tensor_copy(out=maskf, in_=mask_s)
            maskb = smalls.tile([128, J], F32, tag="maskb")
            nc.scalar.activation(
                out=maskb, in_=maskf, func=Copy, bias=-1.0e9, scale=1.0e9
            )
            # PSUM rule: TT may read only one operand from PSUM, so chain
            # (hi + maskb) then (+ lo).
            scores4 = scores_ps[:, 0:100].rearrange("p (t c) -> p t c", c=4)
            scores_hl = smalls.tile([128, J], F32, tag="scores_hl")
            nc.vector.tensor_add(
                scores_hl.rearrange("p (t q) -> p t q", q=2),
                scores4[:, :, 0:2],
                maskb.rearrange("p (t q) -> p t q", q=2),
            )
            scores_sb = smalls.tile([128, J], F32, tag="scores_sb")
            nc.vector.tensor_add(
                scores_sb.rearrange("p (t q) -> p t q", q=2),
                scores4[:, :, 2:4],
                scores_hl.rearrange("p (t q) -> p t q", q=2),
            )
            attn_sb = smalls.tile([128, J], F32, tag="attn_sb")
            den_part = smalls.tile([128, 1], F32, tag="den_part")
            nc.scalar.activation(
                out=attn_sb, in_=scores_sb, func=Exp, accum_out=den_part
            )
            # den[b] = sum of the 4 per-partition partials of batch b (f32 matmul)
            # den col shares the scores bank (already zeroed): start=False
            nc.tensor.matmul(
                scores_ps[0:SLAB_B, 103:104],
                oh4tf_sb,
                den_part,
                start=False,
                stop=True,
                skip_group_check=True,
            )
            recip = smalls.tile([SLAB_B, 1], F32, tag="recip")
            nc.vector.reciprocal(out=recip, in_=scores_ps[0:SLAB_B, 103:104])

            # attn_sel[p, j, m] = attn[p, j] * (p//4 == m)
            attn_sel = qkpool.tile([128, J, SLAB_B], BF16, tag="attn_sel")
            nc.vector.tensor_mul(
                attn_sel,
                oh4t_sb.unsqueeze(1).broadcast_to([128, J, SLAB_B]),
                attn_sb.unsqueeze(2).broadcast_to([128, J, SLAB_B]),
            )

            if stage < 99:
                standin_out(i, attn_sel[0:SLAB_B, 0, :])
                continue

            # phase C: context accumulation over j
            ctx_ps = ctxps_pool.tile([SLAB_B, EMBED], F32, tag="ctxps")
            for j in range(J):
                nc.tensor.matmul(
                    ctx_ps,
                    attn_sel[:, j, :],
                    vals_s[:, j, :],
                    start=(j == 0),
                    stop=(j == J - 1),
                )
            ctx_sb = outpool.tile([SLAB_B, EMBED], F32, tag="ctx_sb")
            nc.scalar.activation(
                out=ctx_sb, in_=ctx_ps, func=Copy, bias=0.0, scale=recip
            )
            nc.sync.dma_start(
                out=out_d[i * SLAB_B : (i + 1) * SLAB_B, :], in_=ctx_sb
            )

    nc.compile()
    return nc


_NC_CACHE = {}


def _get_program(b_core, num_devices):
    key = (b_core, num_devices)
    if key not in _NC_CACHE:
        _NC_CACHE[key] = build_program(b_core, num_devices)
    return _NC_CACHE[key]


def make_in_maps(query, keys, values, mask, W_q, b_q, W_k, b_k, w_e, n_cores):
    consts = _np_consts(W_q, b_q, W_k, b_k, w_e)
    b = query.shape[0]
    b_core = b // n_cores
    in_maps = []
    for c in range(n_cores):
        lo, hi = c * b_core, (c + 1) * b_core
        m = {
            "query": np.ascontiguousarray(query[lo:hi]),
            "keys": np.ascontiguousarray(keys[lo:hi]),
            "values": np.ascontiguousarray(values[lo:hi]),
            "mask": np.ascontiguousarray(mask[lo:hi]),
        }
        m.update(consts)
        in_maps.append(m)
    return in_maps


def kernel(query, keys, values, mask, W_q, b_q, W_k, b_k, w_e):
    n_cores = 8
    query = np.asarray(query, dtype=np.float32)
    keys = np.asarray(keys, dtype=np.float32)
    values = np.asarray(values, dtype=np.float32)
    mask = np.asarray(mask, dtype=np.int32)
    b = query.shape[0]
    b_core = b // n_cores

    nc = _get_program(b_core, n_cores)
    in_maps = make_in_maps(
        query, keys, values, mask,
        np.asarray(W_q), np.asarray(b_q), np.asarray(W_k), np.asarray(b_k),
        np.asarray(w_e), n_cores,
    )
    last_err = None
    for _attempt in range(3):
        try:
            res = bass_utils.run_bass_kernel_spmd(
                nc, in_maps, core_ids=list(range(n_cores))
            )
            break
        except Exception as e:  # transient NRT/device hiccups: retry
            last_err = e
    else:
        raise last_err
    out = np.concatenate([r["out"] for r in res.results], axis=0)
    return out.astype(np.float32)


if __name__ == "__main__":
    rng = np.random.default_rng(0)
    B = 4096
    q = rng.standard_normal((B, EMBED), dtype=np.float32)
    k = rng.standard_normal((B, SEQ, EMBED), dtype=np.float32)
    v = rng.standard_normal((B, SEQ, EMBED), dtype=np.float32)
    msk = np.ones((B, SEQ), dtype=np.int32)
    Wq = rng.standard_normal((EMBED, ATT), dtype=np.float32) * 0.1
    Wk = rng.standard_normal((EMBED, ATT), dtype=np.float32) * 0.1
    bq = np.zeros(ATT, np.float32)
    bk = np.zeros(ATT, np.float32)
    we = rng.standard_normal(ATT, dtype=np.float32) * 0.17
    out = kernel(q, k, v, msk, Wq, bq, Wk, bk, we)
    print(out.shape, out.dtype, out[:2, :4])
